# revision 1
# baseline (speedup 1.0000x reference)
"""Trainium2 Bass kernel for nn_Encoder_29454885716713.

Reference computation (per batch b of B=32, S=2048, F=64):
    q = x @ Wq; k = x @ Wk; v = x @ Wv
    a = softmax(q @ k.T, axis=0 over q)       # query-axis softmax
    out = (a @ v) @ Wh

Sharding: data-parallel over batch, 4 batches per core x 8 cores.

Kernel strategy (per core):
  - ONE input DMA: a host-packed blob [128, 4240] holding Wh (replicated
    4x over partition bands), Wq/Wk/Wv (duplicated in both 64-partition
    halves), and the 4 pre-transposed xT batches packed two-per-row-half.
    5 DMAs total (blob + 4 outputs) so no DMA-lane recycling.
  - qT/kT via matmuls contracting over F partitions; v in natural layout
    [k',d] via matmuls with xT slices as the stationary operand.
  - Per k'-tile t (16 of 128): aT_t = [128, S] in PSUM (fp32r matmuls);
    ScalarE exp with fused accum_out emits exp(aT_t) to SBUF plus the
    query-axis softmax denominator Z[k'] as a per-partition scalar
    (softmax over q == free-dim reduction in this layout; fp32 range
    makes max-subtraction unnecessary).
  - 1/Z folded into v rows; h1 accumulates in four persistent PSUM
    banks, interleaved tile-by-tile into the exp chain (tile t-1's four
    q-chunk matmuls issue during tile t's score matmuls, so the PE work
    hides under ScalarE's exp throughput, the kernel bottleneck);
    out = h1 @ Wh lands output s-tiles in natural layout.
  - This walrus build allows only ONE sync-wait slot per ISA
    instruction.  Tiny per-engine "absorber" ops (dummy matmul / copy /
    nop), each carrying exactly one cross-engine wait, precede any
    instruction that would otherwise need two.
"""

import numpy as np

_CACHE = {}

B, S, F = 32, 2048, 64
DQ, DK, DV = 24, 24, 32
NCORES = 8
BPC = B // NCORES
NT = S // 128
NQC = S // 512

C_WHR = 0
C_WQ = 64
C_WK = 88
C_WV = 112
C_XT = 144
BLOB_COLS = C_XT + (BPC // 2) * S  # 4240


def _build(lowering=True, repeat=1):
    import concourse.bass as bass
    import concourse.mybir as mybir
    import concourse.tile as tile
    from concourse.bass import _add_dep_helper

    f32 = mybir.dt.float32
    f32r = mybir.dt.float32r
    bf16 = mybir.dt.bfloat16
    EXPF = mybir.ActivationFunctionType.Exp

    def r(ap):
        return ap.bitcast(f32r)

    nc = bass.Bass(target_bir_lowering=lowering)
    blob_h = nc.dram_tensor("blob", [128, BLOB_COLS], f32r, kind="ExternalInput")
    out_h = nc.dram_tensor("out", [BPC, S, F], f32, kind="ExternalOutput")
    out_d = out_h.ap()

    with tile.TileContext(nc) as tc:
        with (
            tc.tile_pool(name="consts", bufs=1) as consts,
            tc.tile_pool(name="qkv", bufs=1) as qkv_pool,
            tc.tile_pool(name="vnat", bufs=2) as vnat_pool,
            tc.tile_pool(name="ea", bufs=16) as ea_pool,
            tc.tile_pool(name="zz", bufs=64 * repeat) as z_pool,
            tc.tile_pool(name="vs", bufs=16) as vs_pool,
            tc.tile_pool(name="h1c", bufs=2) as h1c_pool,
            tc.tile_pool(name="ob", bufs=2) as ob_pool,
            tc.tile_pool(name="scr", bufs=1) as scr_pool,
            tc.tile_pool(name="pa", bufs=2, space="PSUM") as pa_pool,
            tc.tile_pool(name="php", bufs=1, space="PSUM") as php_pool,
        ):
            blob_sb = consts.tile([128, BLOB_COLS], f32r)
            sp = C_XT + S
            blob_dma = nc.sync.dma_start(
                out=blob_sb[:, 0:sp], in_=blob_h.ap()[:, 0:sp]
            )
            blob_dma2 = nc.sync.dma_start(
                out=blob_sb[:, sp:BLOB_COLS], in_=blob_h.ap()[:, sp:BLOB_COLS]
            )
            wh_sb = blob_sb[0:DV, 0:F]

            # ---------- absorber machinery ----------
            php = php_pool.tile([128, 4, 512], f32)
            dve_scr = scr_pool.tile([1, 256], f32)
            act_scr = scr_pool.tile([1, 256], f32)
            ctr = {"pe": 0, "dve": 0, "act": 0}

            def pe_absorb(producer):
                c = ctr["pe"]; ctr["pe"] = c + 1
                d = nc.tensor.matmul(
                    php[64:64 + DQ, c % 4, 2 * (c // 4):2 * (c // 4) + 2],
                    blob_sb[64:128, 0:12].bitcast(bf16),
                    blob_sb[64:128, 0:1].bitcast(bf16),
                    start=True, stop=True, skip_group_check=True,
                    tile_position=(64, 64),
                )
                if producer is not None:
                    _add_dep_helper(d.ins, producer.ins, True, "absorb")
                return d

            def dve_absorb(producer):
                c = ctr["dve"] % 250; ctr["dve"] += 1
                d = nc.vector.memset(dve_scr[:, c + 1:c + 2], 0.0)
                _add_dep_helper(d.ins, producer.ins, True, "absorb")
                return d

            def act_absorb(producer):
                c = ctr["act"] % 250; ctr["act"] += 1
                d = nc.scalar.copy(act_scr[:, c + 1:c + 2], act_scr[:, 0:1])
                if producer is not None:
                    _add_dep_helper(d.ins, producer.ins, True, "absorb")
                return d

            def order(after, before):
                _add_dep_helper(after.ins, before.ins, False, "order")

            wfence = pe_absorb(None)  # absorbs blob-DMA-1 wait on PE
            wfence2 = pe_absorb(blob_dma2)  # second half (xT batches 2,3)
            nc.vector.memset(act_scr[:, 0:1], 0.0)
            act_absorb(None)  # ACT observes the act_scr init (DVE) once

            st = {"prev_dve": None}
            pending_s4 = []
            prev_s2 = None
            prev_batch_exp = None
            out_dmas = []
            for rep in range(repeat):
              for b in range(BPC):
                ob = out_d[b].rearrange("(t p) f -> p t f", p=128)
                rb = (b % 2) * 64
                x0 = C_XT + (b // 2) * S
                xT = blob_sb[rb:rb + 64, x0:x0 + S]
                tp = (rb, 0)
                wq_sb = blob_sb[rb:rb + 64, C_WQ:C_WQ + DQ]
                wk_sb = blob_sb[rb:rb + 64, C_WK:C_WK + DK]
                wv_sb = blob_sb[rb:rb + 64, C_WV:C_WV + DV]

                dfence = (
                    pe_absorb(st["prev_dve"])
                    if st["prev_dve"] is not None else None
                )
                s2fence = (
                    pe_absorb(prev_batch_exp)
                    if prev_batch_exp is not None else None
                )
                bfence = (
                    act_absorb(prev_batch_exp)
                    if prev_batch_exp is not None else None
                )

                # ---- qT/kT ----
                qkT = qkv_pool.tile([DV, 2 * S], f32)
                qT = qkT[0:DQ, 0:S]
                kT = qkT[0:DK, S:2 * S]
                for qc in range(NQC):
                    sl = slice(qc * 512, (qc + 1) * 512)
                    for w_sb, dst, dd in ((wq_sb, qT, DQ), (wk_sb, kT, DK)):
                        p = pa_pool.tile([128, 1024], f32, tag="pa")
                        mm = nc.tensor.matmul(
                            p[0:dd, 0:512], w_sb, xT[:, sl],
                            start=True, stop=True, tile_position=tp,
                        )
                        if qc == 0:
                            order(mm, wfence2 if b >= 2 else wfence)
                            if dfence is not None:
                                order(mm, dfence)
                            if s2fence is not None:
                                order(mm, s2fence)
                        da = dve_absorb(mm)
                        cp = nc.vector.tensor_copy(r(dst[:, sl]), p[0:dd, 0:512])
                        order(cp, da)

                # ---- v natural ----
                vnat = vnat_pool.tile([128, NT * DV], f32)
                for g in range(2):
                    pvt = pa_pool.tile([128, 1024], f32, tag="pa")
                    mm = None
                    for i in range(8):
                        t = 8 * g + i
                        mm = nc.tensor.matmul(
                            pvt[:, i * DV:(i + 1) * DV],
                            xT[:, t * 128:(t + 1) * 128], wv_sb,
                            start=True, stop=True, tile_position=tp,
                        )
                        if t == 0:
                            order(mm, wfence)
                    da = dve_absorb(mm)
                    cp = nc.vector.tensor_copy(
                        vnat[:, g * 8 * DV:(g + 1) * 8 * DV], pvt[:, 0:8 * DV]
                    )
                    order(cp, da)
                    last_s2_copy = cp
                prev_s2 = last_s2_copy

                while pending_s4:
                    fn, args = pending_s4.pop(0)
                    fn(*args)
                # ---- aT + exp + Z ----
                qfence = pe_absorb(
                    st["prev_dve"] if st["prev_dve"] is not None else last_s2_copy
                )
                ea_tiles = []
                vs_tiles = []
                prev_exp_h = [None, None]
                last_exp = None
                last_vs = None
                vs_muls = [None] * NT
                for t in range(NT):
                    ea = ea_pool.tile([128, S], f32r, tag="ea")
                    zp = z_pool.tile([128, 2], f32, tag="zp")
                    for h in range(2):
                        afence = (
                            pe_absorb(prev_exp_h[h])
                            if prev_exp_h[h] is not None else None
                        )
                        pa = pa_pool.tile([128, 1024], f32, tag="pa")
                        lastmm = None
                        for j in range(2):
                            qc = 2 * h + j
                            lastmm = nc.tensor.matmul(
                                pa[:, j * 512:(j + 1) * 512],
                                r(kT[:, t * 128:(t + 1) * 128]),
                                r(qT[:, qc * 512:(qc + 1) * 512]),
                                start=True, stop=True,
                            )
                            if afence is not None:
                                order(lastmm, afence)
                            if t == 0:
                                order(lastmm, qfence)
                        if h == 1 and t >= 1:
                            # interleave h1 accumulation for tile t-1, all qc
                            vf = pe_absorb(vs_muls[t - 1])
                            for qc2 in range(4):
                                hm = nc.tensor.matmul(
                                    php[0:DV, qc2, :],
                                    vs_tiles[t - 1],
                                    ea_tiles[t - 1][:, qc2 * 512:(qc2 + 1) * 512],
                                    start=(t - 1 == 0), stop=False,
                                    skip_group_check=True,
                                )
                                order(hm, afence)
                                order(hm, vf)
                        last_exp = nc.scalar.activation(
                            out=ea[:, h * 1024:(h + 1) * 1024],
                            in_=pa,
                            func=EXPF,
                            accum_out=zp[:, h:h + 1],
                        )
                        prev_exp_h[h] = last_exp
                        if bfence is not None:
                            order(last_exp, bfence)
                    zs = z_pool.tile([128, 1], f32, tag="zs")
                    nc.vector.tensor_add(zs, zp[:, 0:1], zp[:, 1:2])
                    zi = z_pool.tile([128, 1], f32, tag="zi")
                    nc.vector.reciprocal(zi, zs)
                    vs = vs_pool.tile([128, DV], f32r, tag="vs")
                    last_vs = nc.vector.tensor_scalar_mul(
                        vs, vnat[:, t * DV:(t + 1) * DV], zi
                    )
                    ea_tiles.append(ea)
                    vs_tiles.append(vs)
                    vs_muls[t] = last_vs
                prev_batch_exp = last_exp

                def s4(ea_tiles, vs_tiles, vs_muls, last_exp, last_vs, ob):
                    # ---- h1 + out ----
                    hfence_a = pe_absorb(last_exp)
                    hfence_d = pe_absorb(last_vs)
                    obuf = ob_pool.tile([128, NT * F], f32)
                    for qc in range(NQC):
                        sl = slice(qc * 512, (qc + 1) * 512)
                        lastmm = nc.tensor.matmul(
                            php[0:DV, qc, :],
                            vs_tiles[NT - 1],
                            ea_tiles[NT - 1][:, sl],
                            start=False, stop=True,
                            skip_group_check=True,
                        )
                        order(lastmm, hfence_a)
                        order(lastmm, hfence_d)
                        ph1 = php[0:DV, qc, :]
                        da = dve_absorb(lastmm)
                        h1cat = h1c_pool.tile([DV, 512], f32r)
                        h1copy = nc.vector.tensor_copy(h1cat, ph1)
                        order(h1copy, da)
                        ofence = pe_absorb(h1copy)
                        pout = pa_pool.tile([128, 1024], f32, tag="pa")
                        lastmm = None
                        for si in range(4):
                            lastmm = nc.tensor.matmul(
                                pout[:, si * F:(si + 1) * F],
                                h1cat[:, si * 128:(si + 1) * 128],
                                wh_sb,
                                start=True, stop=True,
                            )
                            order(lastmm, ofence)
                        da = dve_absorb(lastmm)
                        if len(out_dmas) >= 2:
                            # obuf slot reuse: absorb the old out-DMA's tick
                            da2 = dve_absorb(out_dmas[-2])
                        else:
                            da2 = None
                        st["prev_dve"] = nc.vector.tensor_copy(
                            obuf[:, qc * 4 * F:(qc + 1) * 4 * F], pout[:, 0:4 * F]
                        )
                        order(st["prev_dve"], da)
                        if da2 is not None:
                            order(st["prev_dve"], da2)
                    if rep == 0:
                        odma = nc.sync.dma_start(
                            out=ob, in_=obuf.rearrange("p (t f) -> p t f", f=F)
                        )
                        out_dmas.append(odma)

                pending_s4.append((s4, (ea_tiles, vs_tiles, vs_muls, last_exp, last_vs, ob)))
            while pending_s4:
                fn, args = pending_s4.pop(0)
                fn(*args)
            # ---- tail: sync-nop chain so the auto drain keeps <=1 wait ----
            for fin in [st["prev_dve"], prev_batch_exp, blob_dma, blob_dma2] + out_dmas:
                if fin is None:
                    continue
                n = nc.sync.nop()
                _add_dep_helper(n.ins, fin.ins, True, "drain pre-wait")
    return nc


def _get_nc():
    if "nc" not in _CACHE:
        _CACHE["nc"] = _build()
    return _CACHE["nc"]


def make_in_maps(x, Wq, Wk, Wv, Wh):
    x = np.asarray(x, dtype=np.float32)
    xt = np.ascontiguousarray(x.transpose(0, 2, 1))  # [B, F, S]
    wq = np.asarray(Wq, dtype=np.float32)
    wk = np.asarray(Wk, dtype=np.float32)
    wv = np.asarray(Wv, dtype=np.float32)
    base = np.zeros((128, BLOB_COLS), dtype=np.float32)
    base[0:DV, 0:F] = np.asarray(Wh, dtype=np.float32)
    for rb in (0, 64):
        base[rb:rb + 64, C_WQ:C_WQ + DQ] = wq
        base[rb:rb + 64, C_WK:C_WK + DK] = wk
        base[rb:rb + 64, C_WV:C_WV + DV] = wv
    maps = []
    for i in range(NCORES):
        blob = base.copy()
        for b in range(BPC):
            rb = (b % 2) * 64
            x0 = C_XT + (b // 2) * S
            blob[rb:rb + 64, x0:x0 + S] = xt[i * BPC + b]
        maps.append({"blob": blob})
    return maps


def kernel(x, Wq, Wk, Wv, Wh):
    from concourse.bass_utils import run_bass_kernel_spmd

    nc = _get_nc()
    in_maps = make_in_maps(x, Wq, Wk, Wv, Wh)
    res = run_bass_kernel_spmd(nc, in_maps, core_ids=list(range(NCORES)))
    out = np.concatenate([res.results[i]["out"] for i in range(NCORES)], axis=0)
    return out



# revision 3
# speedup vs baseline: 19.9562x; 19.9562x over previous
"""Trainium2 Bass kernel for nn_Encoder_29454885716713.

Reference computation (per batch b of B=32, S=2048, F=64):
    q = x @ Wq; k = x @ Wk; v = x @ Wv
    a = softmax(q @ k.T, axis=0 over q)       # query-axis softmax
    out = (a @ v) @ Wh

Sharding: data-parallel over batch, 4 batches per core x 8 cores.

Kernel strategy (per core):
  - ONE input DMA: a host-packed blob [128, 4240] holding Wh (replicated
    4x over partition bands), Wq/Wk/Wv (duplicated in both 64-partition
    halves), and the 4 pre-transposed xT batches packed two-per-row-half.
    5 DMAs total (blob + 4 outputs) so no DMA-lane recycling.
  - qT/kT via matmuls contracting over F partitions; v in natural layout
    [k',d] via matmuls with xT slices as the stationary operand.
  - Per k'-tile t (16 of 128): aT_t = [128, S] in PSUM (fp32r matmuls);
    ScalarE exp with fused accum_out emits exp(aT_t) to SBUF plus the
    query-axis softmax denominator Z[k'] as a per-partition scalar
    (softmax over q == free-dim reduction in this layout; fp32 range
    makes max-subtraction unnecessary).
  - 1/Z folded into v rows; h1 accumulates in four persistent PSUM
    banks, interleaved tile-by-tile into the exp chain (tile t-1's four
    q-chunk matmuls issue during tile t's score matmuls, so the PE work
    hides under ScalarE's exp throughput, the kernel bottleneck);
    out = h1 @ Wh lands output s-tiles in natural layout.
  - This walrus build allows only ONE sync-wait slot per ISA
    instruction.  Tiny per-engine "absorber" ops (dummy matmul / copy /
    nop), each carrying exactly one cross-engine wait, precede any
    instruction that would otherwise need two.
"""

import numpy as np

_CACHE = {}

B, S, F = 32, 2048, 64
DQ, DK, DV = 24, 24, 32
NCORES = 8
BPC = B // NCORES
NT = S // 128
NQC = S // 512

C_WHR = 0
C_WQ = 64
C_WK = 88
C_WV = 112
C_XT = 144
BLOB_COLS = C_XT + (BPC // 2) * S  # 4240


def _build(lowering=True, repeat=1):
    import concourse.bass as bass
    import concourse.mybir as mybir
    import concourse.tile as tile
    from concourse.bass import _add_dep_helper

    f32 = mybir.dt.float32
    f32r = mybir.dt.float32r
    bf16 = mybir.dt.bfloat16
    EXPF = mybir.ActivationFunctionType.Exp

    def r(ap):
        return ap.bitcast(f32r)

    nc = bass.Bass(target_bir_lowering=lowering)
    blob_h = nc.dram_tensor("blob", [128, BLOB_COLS], f32r, kind="ExternalInput")
    out_h = nc.dram_tensor("out", [BPC, S, F], f32, kind="ExternalOutput")
    out_d = out_h.ap()

    with tile.TileContext(nc) as tc:
        with (
            tc.tile_pool(name="consts", bufs=1) as consts,
            tc.tile_pool(name="qkv", bufs=1) as qkv_pool,
            tc.tile_pool(name="vnat", bufs=2) as vnat_pool,
            tc.tile_pool(name="ea", bufs=16) as ea_pool,
            tc.tile_pool(name="zz", bufs=64 * repeat) as z_pool,
            tc.tile_pool(name="vs", bufs=16) as vs_pool,
            tc.tile_pool(name="h1c", bufs=2) as h1c_pool,
            tc.tile_pool(name="ob", bufs=2) as ob_pool,
            tc.tile_pool(name="scr", bufs=1) as scr_pool,
            tc.tile_pool(name="pa", bufs=2, space="PSUM") as pa_pool,
            tc.tile_pool(name="php", bufs=1, space="PSUM") as php_pool,
        ):
            blob_sb = consts.tile([128, BLOB_COLS], f32r)
            sp = C_XT + S
            blob_dma = nc.sync.dma_start(
                out=blob_sb[:, 0:sp], in_=blob_h.ap()[:, 0:sp]
            )
            blob_dma2 = nc.sync.dma_start(
                out=blob_sb[:, sp:BLOB_COLS], in_=blob_h.ap()[:, sp:BLOB_COLS]
            )
            wh_sb = blob_sb[0:DV, 0:F]

            # ---------- absorber machinery ----------
            php = php_pool.tile([128, 4, 512], f32)
            dve_scr = scr_pool.tile([1, 256], f32)
            act_scr = scr_pool.tile([1, 256], f32)
            ctr = {"pe": 0, "dve": 0, "act": 0}

            def pe_absorb(producer):
                c = ctr["pe"]; ctr["pe"] = c + 1
                d = nc.tensor.matmul(
                    php[64:64 + DQ, c % 4, 2 * (c // 4):2 * (c // 4) + 2],
                    blob_sb[64:128, 0:12].bitcast(bf16),
                    blob_sb[64:128, 0:1].bitcast(bf16),
                    start=True, stop=True, skip_group_check=True,
                    tile_position=(64, 64),
                )
                if producer is not None:
                    _add_dep_helper(d.ins, producer.ins, True, "absorb")
                return d

            def dve_absorb(producer):
                c = ctr["dve"] % 250; ctr["dve"] += 1
                d = nc.vector.memset(dve_scr[:, c + 1:c + 2], 0.0)
                _add_dep_helper(d.ins, producer.ins, True, "absorb")
                return d

            def act_absorb(producer):
                c = ctr["act"] % 250; ctr["act"] += 1
                d = nc.scalar.copy(act_scr[:, c + 1:c + 2], act_scr[:, 0:1])
                if producer is not None:
                    _add_dep_helper(d.ins, producer.ins, True, "absorb")
                return d

            def order(after, before):
                _add_dep_helper(after.ins, before.ins, False, "order")

            wfence = pe_absorb(None)  # absorbs blob-DMA-1 wait on PE
            wfence2 = pe_absorb(blob_dma2)  # second half (xT batches 2,3)
            nc.vector.memset(act_scr[:, 0:1], 0.0)
            act_absorb(None)  # ACT observes the act_scr init (DVE) once

            st = {"prev_dve": None}
            pending_s4 = []
            prev_s2 = None
            prev_batch_exp = None
            out_dmas = []
            for rep in range(repeat):
              for b in range(BPC):
                ob = out_d[b].rearrange("(t p) f -> p t f", p=128)
                rb = (b % 2) * 64
                x0 = C_XT + (b // 2) * S
                xT = blob_sb[rb:rb + 64, x0:x0 + S]
                tp = (rb, 0)
                wq_sb = blob_sb[rb:rb + 64, C_WQ:C_WQ + DQ]
                wk_sb = blob_sb[rb:rb + 64, C_WK:C_WK + DK]
                wv_sb = blob_sb[rb:rb + 64, C_WV:C_WV + DV]

                dfence = (
                    pe_absorb(st["prev_dve"])
                    if st["prev_dve"] is not None else None
                )
                s2fence = (
                    pe_absorb(prev_batch_exp)
                    if prev_batch_exp is not None else None
                )
                bfence = (
                    act_absorb(prev_batch_exp)
                    if prev_batch_exp is not None else None
                )

                # ---- qT/kT ----
                qkT = qkv_pool.tile([DV, 2 * S], f32)
                qT = qkT[0:DQ, 0:S]
                kT = qkT[0:DK, S:2 * S]
                for qc in range(NQC):
                    sl = slice(qc * 512, (qc + 1) * 512)
                    for w_sb, dst, dd in ((wq_sb, qT, DQ), (wk_sb, kT, DK)):
                        p = pa_pool.tile([128, 1024], f32, tag="pa")
                        mm = nc.tensor.matmul(
                            p[0:dd, 0:512], w_sb, xT[:, sl],
                            start=True, stop=True, tile_position=tp,
                        )
                        if qc == 0:
                            order(mm, wfence2 if b >= 2 else wfence)
                            if dfence is not None:
                                order(mm, dfence)
                            if s2fence is not None:
                                order(mm, s2fence)
                        da = dve_absorb(mm)
                        cp = nc.vector.tensor_copy(r(dst[:, sl]), p[0:dd, 0:512])
                        order(cp, da)

                # ---- v natural ----
                vnat = vnat_pool.tile([128, NT * DV], f32)
                for g in range(2):
                    pvt = pa_pool.tile([128, 1024], f32, tag="pa")
                    mm = None
                    for i in range(8):
                        t = 8 * g + i
                        mm = nc.tensor.matmul(
                            pvt[:, i * DV:(i + 1) * DV],
                            xT[:, t * 128:(t + 1) * 128], wv_sb,
                            start=True, stop=True, tile_position=tp,
                        )
                        if t == 0:
                            order(mm, wfence)
                    da = dve_absorb(mm)
                    cp = nc.vector.tensor_copy(
                        vnat[:, g * 8 * DV:(g + 1) * 8 * DV], pvt[:, 0:8 * DV]
                    )
                    order(cp, da)
                    last_s2_copy = cp
                prev_s2 = last_s2_copy

                while pending_s4:
                    fn, args = pending_s4.pop(0)
                    fn(*args)
                # ---- aT + exp + Z ----
                qfence = pe_absorb(
                    st["prev_dve"] if st["prev_dve"] is not None else last_s2_copy
                )
                ea_tiles = []
                vs_tiles = []
                prev_exp_h = [None, None]
                last_exp = None
                last_vs = None
                vs_muls = [None] * NT
                for t in range(NT):
                    ea = ea_pool.tile([128, S], bf16, tag="ea")
                    zp = z_pool.tile([128, 2], f32, tag="zp")
                    for h in range(2):
                        afence = (
                            pe_absorb(prev_exp_h[h])
                            if prev_exp_h[h] is not None else None
                        )
                        pa = pa_pool.tile([128, 1024], f32, tag="pa")
                        lastmm = None
                        for j in range(2):
                            qc = 2 * h + j
                            lastmm = nc.tensor.matmul(
                                pa[:, j * 512:(j + 1) * 512],
                                r(kT[:, t * 128:(t + 1) * 128]),
                                r(qT[:, qc * 512:(qc + 1) * 512]),
                                start=True, stop=True,
                            )
                            if afence is not None:
                                order(lastmm, afence)
                            if t == 0:
                                order(lastmm, qfence)
                        if h == 1 and t >= 1:
                            # interleave h1 accumulation for tile t-1, all qc
                            vf = pe_absorb(vs_muls[t - 1])
                            for qc2 in range(4):
                                hm = nc.tensor.matmul(
                                    php[0:DV, qc2, :],
                                    vs_tiles[t - 1],
                                    ea_tiles[t - 1][:, qc2 * 512:(qc2 + 1) * 512],
                                    start=(t - 1 == 0), stop=False,
                                    skip_group_check=True,
                                )
                                order(hm, afence)
                                order(hm, vf)
                        last_exp = nc.scalar.activation(
                            out=ea[:, h * 1024:(h + 1) * 1024],
                            in_=pa,
                            func=EXPF,
                            accum_out=zp[:, h:h + 1],
                        )
                        prev_exp_h[h] = last_exp
                        if bfence is not None:
                            order(last_exp, bfence)
                    zs = z_pool.tile([128, 1], f32, tag="zs")
                    nc.vector.tensor_add(zs, zp[:, 0:1], zp[:, 1:2])
                    zi = z_pool.tile([128, 1], f32, tag="zi")
                    nc.vector.reciprocal(zi, zs)
                    vs = vs_pool.tile([128, DV], bf16, tag="vs")
                    last_vs = nc.vector.tensor_scalar_mul(
                        vs, vnat[:, t * DV:(t + 1) * DV], zi
                    )
                    ea_tiles.append(ea)
                    vs_tiles.append(vs)
                    vs_muls[t] = last_vs
                prev_batch_exp = last_exp

                def s4(ea_tiles, vs_tiles, vs_muls, last_exp, last_vs, ob):
                    # ---- h1 + out ----
                    hfence_a = pe_absorb(last_exp)
                    hfence_d = pe_absorb(last_vs)
                    obuf = ob_pool.tile([128, NT * F], f32)
                    for qc in range(NQC):
                        sl = slice(qc * 512, (qc + 1) * 512)
                        lastmm = nc.tensor.matmul(
                            php[0:DV, qc, :],
                            vs_tiles[NT - 1],
                            ea_tiles[NT - 1][:, sl],
                            start=False, stop=True,
                            skip_group_check=True,
                        )
                        order(lastmm, hfence_a)
                        order(lastmm, hfence_d)
                        ph1 = php[0:DV, qc, :]
                        da = dve_absorb(lastmm)
                        h1cat = h1c_pool.tile([DV, 512], f32r)
                        h1copy = nc.vector.tensor_copy(h1cat, ph1)
                        order(h1copy, da)
                        ofence = pe_absorb(h1copy)
                        pout = pa_pool.tile([128, 1024], f32, tag="pa")
                        lastmm = None
                        for si in range(4):
                            lastmm = nc.tensor.matmul(
                                pout[:, si * F:(si + 1) * F],
                                h1cat[:, si * 128:(si + 1) * 128],
                                wh_sb,
                                start=True, stop=True,
                            )
                            order(lastmm, ofence)
                        da = dve_absorb(lastmm)
                        if len(out_dmas) >= 2:
                            # obuf slot reuse: absorb the old out-DMA's tick
                            da2 = dve_absorb(out_dmas[-2])
                        else:
                            da2 = None
                        st["prev_dve"] = nc.vector.tensor_copy(
                            obuf[:, qc * 4 * F:(qc + 1) * 4 * F], pout[:, 0:4 * F]
                        )
                        order(st["prev_dve"], da)
                        if da2 is not None:
                            order(st["prev_dve"], da2)
                    if rep == 0:
                        odma = nc.sync.dma_start(
                            out=ob, in_=obuf.rearrange("p (t f) -> p t f", f=F)
                        )
                        out_dmas.append(odma)

                pending_s4.append((s4, (ea_tiles, vs_tiles, vs_muls, last_exp, last_vs, ob)))
            while pending_s4:
                fn, args = pending_s4.pop(0)
                fn(*args)
            # ---- tail: sync-nop chain so the auto drain keeps <=1 wait ----
            for fin in [st["prev_dve"], prev_batch_exp, blob_dma, blob_dma2] + out_dmas:
                if fin is None:
                    continue
                n = nc.sync.nop()
                _add_dep_helper(n.ins, fin.ins, True, "drain pre-wait")
    return nc


def _get_nc():
    if "nc" not in _CACHE:
        _CACHE["nc"] = _build()
    return _CACHE["nc"]


def make_in_maps(x, Wq, Wk, Wv, Wh):
    x = np.asarray(x, dtype=np.float32)
    xt = np.ascontiguousarray(x.transpose(0, 2, 1))  # [B, F, S]
    wq = np.asarray(Wq, dtype=np.float32)
    wk = np.asarray(Wk, dtype=np.float32)
    wv = np.asarray(Wv, dtype=np.float32)
    base = np.zeros((128, BLOB_COLS), dtype=np.float32)
    base[0:DV, 0:F] = np.asarray(Wh, dtype=np.float32)
    for rb in (0, 64):
        base[rb:rb + 64, C_WQ:C_WQ + DQ] = wq
        base[rb:rb + 64, C_WK:C_WK + DK] = wk
        base[rb:rb + 64, C_WV:C_WV + DV] = wv
    maps = []
    for i in range(NCORES):
        blob = base.copy()
        for b in range(BPC):
            rb = (b % 2) * 64
            x0 = C_XT + (b // 2) * S
            blob[rb:rb + 64, x0:x0 + S] = xt[i * BPC + b]
        maps.append({"blob": blob})
    return maps


def kernel(x, Wq, Wk, Wv, Wh):
    from concourse.bass_utils import run_bass_kernel_spmd

    nc = _get_nc()
    in_maps = make_in_maps(x, Wq, Wk, Wv, Wh)
    res = run_bass_kernel_spmd(nc, in_maps, core_ids=list(range(NCORES)))
    out = np.concatenate([res.results[i]["out"] for i in range(NCORES)], axis=0)
    return out



# revision 5
# speedup vs baseline: 20.8680x; 1.0457x over previous
"""Trainium2 Bass kernel for nn_Encoder_29454885716713.

Reference computation (per batch b of B=32, S=2048, F=64):
    q = x @ Wq; k = x @ Wk; v = x @ Wv
    a = softmax(q @ k.T, axis=0 over q)       # query-axis softmax
    out = (a @ v) @ Wh
Sharding: data-parallel over batch, 4 batches per core x 8 cores.

Kernel strategy (per core) — ACT(exp)-bound, fully batch-pipelined:
  - ONE input DMA blob [128, 4240]: Wh replicated on all four 32-row
    bands, Wq/Wk/Wv duplicated in both 64-row halves, 4 pre-transposed
    xT batches packed two-per-row-half.
  - qkv stage runs in its own 1-bank PSUM pool (pq) so it overlaps the
    previous batch's score/exp chain instead of serializing behind it;
    qkT and vnat are double-buffered across batches.
  - Per k'-tile t (16 of 128): aT_t = [128, S] in PSUM (fp32r matmuls);
    ScalarE exp with fused accum_out emits exp(aT_t) to SBUF (bf16)
    plus the query-axis softmax denominator Z[k'] (free-dim reduction;
    fp32 range makes max-subtraction unnecessary).
  - 1/Z folded into v rows (vs, bf16); h1 accumulates in ONE persistent
    PSUM bank with the four 512-wide q-chunks packed at partition bases
    0/32/64/96 (PE tile_position col steps of 32), interleaved
    tile-by-tile into the exp chain; out = h1 @ Wh uses the Wh copy on
    the matching partition band, all 16 s-tiles into one PSUM tile,
    one obuf copy, one DMA per batch.
  - PSUM budget: scores 2x[128,1024] (4 banks) + pq 2x[128,512] (2)
    + h1 bank (1) + absorber bank (1) = 8.
  - This walrus build allows only ONE sync-wait slot per ISA
    instruction.  Tiny per-engine "absorber" ops (dummy matmul / copy /
    nop), each carrying exactly one cross-engine wait, precede any
    instruction that would otherwise need two.
"""

import numpy as np

_CACHE = {}

B, S, F = 32, 2048, 64
DQ, DK, DV = 24, 24, 32
NCORES = 8
BPC = B // NCORES
NT = S // 128
NQC = S // 512

C_WHR = 0
C_WQ = 64
C_WK = 88
C_WV = 112
C_XT = 144
BLOB_COLS = C_XT + (BPC // 2) * S  # 4240


def _build(lowering=True, repeat=1):
    import concourse.bass as bass
    import concourse.mybir as mybir
    import concourse.tile as tile
    from concourse.bass import _add_dep_helper

    f32 = mybir.dt.float32
    f32r = mybir.dt.float32r
    bf16 = mybir.dt.bfloat16
    EXPF = mybir.ActivationFunctionType.Exp

    def r(ap):
        return ap.bitcast(f32r)

    nc = bass.Bass(target_bir_lowering=lowering)
    blob_h = nc.dram_tensor("blob", [128, BLOB_COLS], f32r, kind="ExternalInput")
    out_h = nc.dram_tensor("out", [BPC, S, F], f32, kind="ExternalOutput")
    out_d = out_h.ap()

    with tile.TileContext(nc) as tc:
        with (
            tc.tile_pool(name="consts", bufs=1) as consts,
            tc.tile_pool(name="qkv", bufs=2) as qkv_pool,
            tc.tile_pool(name="vnat", bufs=2) as vnat_pool,
            tc.tile_pool(name="ea", bufs=16) as ea_pool,
            tc.tile_pool(name="zz", bufs=64) as z_pool,
            tc.tile_pool(name="vs", bufs=16) as vs_pool,
            tc.tile_pool(name="h1c", bufs=2) as h1c_pool,
            tc.tile_pool(name="ob", bufs=2) as ob_pool,
            tc.tile_pool(name="scr", bufs=1) as scr_pool,
            tc.tile_pool(name="pa", bufs=2, space="PSUM") as pa_pool,
            tc.tile_pool(name="pq", bufs=2, space="PSUM") as pq_pool,
            tc.tile_pool(name="php", bufs=1, space="PSUM") as php_pool,
        ):
            blob_sb = consts.tile([128, BLOB_COLS], f32r)
            sp = C_XT + S
            blob_dma = nc.sync.dma_start(
                out=blob_sb[:, 0:sp], in_=blob_h.ap()[:, 0:sp]
            )
            blob_dma2 = nc.sync.dma_start(
                out=blob_sb[:, sp:BLOB_COLS], in_=blob_h.ap()[:, sp:BLOB_COLS]
            )

            # ---------- absorber machinery ----------
            php = php_pool.tile([128, 1024], f32)
            dve_scr = scr_pool.tile([1, 256], f32)
            act_scr = scr_pool.tile([1, 256], f32)
            ctr = {"pe": 0, "dve": 0, "act": 0}

            def pe_absorb(producer):
                c = ctr["pe"]; ctr["pe"] = c + 1
                d = nc.tensor.matmul(
                    php[32:32 + DQ, (2 * c) % 1000:(2 * c) % 1000 + 2],
                    blob_sb[64:128, 0:12].bitcast(bf16),
                    blob_sb[64:128, 0:1].bitcast(bf16),
                    start=True, stop=True, skip_group_check=True,
                    tile_position=(64, 32),
                )
                if producer is not None:
                    _add_dep_helper(d.ins, producer.ins, True, "absorb")
                return d

            def dve_absorb(producer):
                c = ctr["dve"] % 250; ctr["dve"] += 1
                d = nc.vector.memset(dve_scr[:, c + 1:c + 2], 0.0)
                _add_dep_helper(d.ins, producer.ins, True, "absorb")
                return d

            def act_absorb(producer):
                c = ctr["act"] % 250; ctr["act"] += 1
                d = nc.scalar.copy(act_scr[:, c + 1:c + 2], act_scr[:, 0:1])
                if producer is not None:
                    _add_dep_helper(d.ins, producer.ins, True, "absorb")
                return d

            def order(after, before):
                _add_dep_helper(after.ins, before.ins, False, "order")

            wfence = pe_absorb(None)  # absorbs blob-DMA-1 wait on PE
            wfence2 = None  # created lazily before batch 2's qkv
            nc.vector.memset(act_scr[:, 0:1], 0.0)
            act_absorb(None)  # ACT observes the act_scr init (DVE) once

            st = {"prev_dve": None}
            pending_s4 = []
            prev_batch_exp = None
            prev_exp_h = [None, None]  # persists across batches
            out_dmas = []
            for rep in range(repeat):
              for b in range(BPC):
                ob = out_d[b].rearrange("(t p) f -> p t f", p=128)
                rb = (b % 2) * 64
                x0 = C_XT + (b // 2) * S
                xT = blob_sb[rb:rb + 64, x0:x0 + S]
                tp = (rb, 0)
                wq_sb = blob_sb[rb:rb + 64, C_WQ:C_WQ + DQ]
                wk_sb = blob_sb[rb:rb + 64, C_WK:C_WK + DK]
                wv_sb = blob_sb[rb:rb + 64, C_WV:C_WV + DV]

                if rep == 0 and b == 2 and wfence2 is None:
                    wfence2 = pe_absorb(blob_dma2)  # second half (xT 2,3)

                dfence = (
                    pe_absorb(st["prev_dve"])
                    if st["prev_dve"] is not None else None
                )

                # ---- qT/kT (pq pool, overlaps prev batch's exp chain) ----
                qkT = qkv_pool.tile([DV, 2 * S], f32)
                qT = qkT[0:DQ, 0:S]
                kT = qkT[0:DK, S:2 * S]
                for qc in range(NQC):
                    sl = slice(qc * 512, (qc + 1) * 512)
                    for w_sb, dst, dd in ((wq_sb, qT, DQ), (wk_sb, kT, DK)):
                        p = pq_pool.tile([128, 512], f32, tag="pq")
                        mm = nc.tensor.matmul(
                            p[0:dd, 0:512], w_sb, xT[:, sl],
                            start=True, stop=True, tile_position=tp,
                        )
                        if qc == 0:
                            order(mm, wfence2 if b >= 2 else wfence)
                            if dfence is not None:
                                order(mm, dfence)
                        da = dve_absorb(mm)
                        cp = nc.vector.tensor_copy(r(dst[:, sl]), p[0:dd, 0:512])
                        order(cp, da)

                # ---- v natural ----
                vnat = vnat_pool.tile([128, NT * DV], f32)
                for g in range(2):
                    pvt = pq_pool.tile([128, 512], f32, tag="pq")
                    mm = None
                    for i in range(8):
                        t = 8 * g + i
                        mm = nc.tensor.matmul(
                            pvt[:, i * DV:(i + 1) * DV],
                            xT[:, t * 128:(t + 1) * 128], wv_sb,
                            start=True, stop=True, tile_position=tp,
                        )
                        if t == 0:
                            order(mm, wfence2 if b >= 2 else wfence)
                    da = dve_absorb(mm)
                    cp = nc.vector.tensor_copy(
                        vnat[:, g * 8 * DV:(g + 1) * 8 * DV], pvt[:, 0:8 * DV]
                    )
                    order(cp, da)
                    last_s2_copy = cp

                # ---- prev batch's tail (h1 final tile + out + DMA) ----
                while pending_s4:
                    fn, args = pending_s4.pop(0)
                    fn(*args)
                # ---- aT + exp + Z ----
                qfence = pe_absorb(
                    st["prev_dve"] if st["prev_dve"] is not None else last_s2_copy
                )
                ea_tiles = []
                vs_tiles = []
                last_exp = None
                last_vs = None
                vs_muls = [None] * NT
                for t in range(NT):
                    ea = ea_pool.tile([128, S], bf16, tag="ea")
                    zp = z_pool.tile([128, 2], f32, tag="zp")
                    for h in range(2):
                        afence = (
                            pe_absorb(prev_exp_h[h])
                            if prev_exp_h[h] is not None else None
                        )
                        pa = pa_pool.tile([128, 1024], f32, tag="pa")
                        lastmm = None
                        for j in range(2):
                            qc = 2 * h + j
                            lastmm = nc.tensor.matmul(
                                pa[:, j * 512:(j + 1) * 512],
                                r(kT[:, t * 128:(t + 1) * 128]),
                                r(qT[:, qc * 512:(qc + 1) * 512]),
                                start=True, stop=True,
                            )
                            if afence is not None:
                                order(lastmm, afence)
                            if t == 0:
                                order(lastmm, qfence)
                        if h == 1 and t >= 1:
                            # interleave h1 accumulation for tile t-1, all qc
                            vf = pe_absorb(vs_muls[t - 1])
                            for qc2 in range(4):
                                rb2 = 64 * (qc2 % 2)
                                cs = 512 * (qc2 // 2)
                                hm = nc.tensor.matmul(
                                    php[rb2:rb2 + DV, cs:cs + 512],
                                    vs_tiles[t - 1],
                                    ea_tiles[t - 1][:, qc2 * 512:(qc2 + 1) * 512],
                                    start=(t - 1 == 0), stop=False,
                                    skip_group_check=True,
                                )
                                order(hm, afence)
                                order(hm, vf)
                        last_exp = nc.scalar.activation(
                            out=ea[:, h * 1024:(h + 1) * 1024],
                            in_=pa,
                            func=EXPF,
                            accum_out=zp[:, h:h + 1],
                        )
                        prev_exp_h[h] = last_exp
                    zs = z_pool.tile([128, 1], f32, tag="zs")
                    nc.vector.tensor_add(zs, zp[:, 0:1], zp[:, 1:2])
                    zi = z_pool.tile([128, 1], f32, tag="zi")
                    nc.vector.reciprocal(zi, zs)
                    vs = vs_pool.tile([128, DV], bf16, tag="vs")
                    last_vs = nc.vector.tensor_scalar_mul(
                        vs, vnat[:, t * DV:(t + 1) * DV], zi
                    )
                    ea_tiles.append(ea)
                    vs_tiles.append(vs)
                    vs_muls[t] = last_vs
                prev_batch_exp = last_exp

                def s4(ea_tiles, vs_tiles, last_exp, last_vs, ob, rep):
                    # ---- h1 final tile + out ----
                    hfence_a = pe_absorb(last_exp)
                    hfence_d = pe_absorb(last_vs)
                    lastmm = None
                    for qc in range(NQC):
                        rb2 = 64 * (qc % 2)
                        cs = 512 * (qc // 2)
                        lastmm = nc.tensor.matmul(
                            php[rb2:rb2 + DV, cs:cs + 512],
                            vs_tiles[NT - 1],
                            ea_tiles[NT - 1][:, qc * 512:(qc + 1) * 512],
                            start=False, stop=True,
                            skip_group_check=True,
                        )
                        order(lastmm, hfence_a)
                        order(lastmm, hfence_d)
                    da = dve_absorb(lastmm)
                    h1cat = h1c_pool.tile([128, 1024], f32r)
                    h1copy = nc.vector.tensor_copy(
                        h1cat[0:DV, :], php[0:DV, :]
                    )
                    order(h1copy, da)
                    h1copy = nc.vector.tensor_copy(
                        h1cat[64:64 + DV, :], php[64:64 + DV, :]
                    )
                    ofence = pe_absorb(h1copy)
                    pout = pa_pool.tile([128, 1024], f32, tag="pa")
                    lastmm = None
                    for qc in range(NQC):
                        rb2 = 64 * (qc % 2)
                        cs = 512 * (qc // 2)
                        wh_q = blob_sb[rb2:rb2 + DV, 0:F]
                        for si in range(4):
                            ti = 4 * qc + si
                            lastmm = nc.tensor.matmul(
                                pout[:, ti * F:(ti + 1) * F],
                                h1cat[rb2:rb2 + DV,
                                      cs + si * 128:cs + (si + 1) * 128],
                                wh_q,
                                start=True, stop=True,
                                tile_position=(rb2, 0),
                            )
                            order(lastmm, ofence)
                    da = dve_absorb(lastmm)
                    obuf = ob_pool.tile([128, NT * F], f32)
                    if len(out_dmas) >= 2:
                        # obuf slot reuse: absorb the old out-DMA's tick
                        da2 = dve_absorb(out_dmas[-2])
                    else:
                        da2 = None
                    st["prev_dve"] = nc.vector.tensor_copy(obuf, pout)
                    order(st["prev_dve"], da)
                    if da2 is not None:
                        order(st["prev_dve"], da2)
                    if rep == 0:
                        odma = nc.sync.dma_start(
                            out=ob, in_=obuf.rearrange("p (t f) -> p t f", f=F)
                        )
                        out_dmas.append(odma)

                pending_s4.append(
                    (s4, (ea_tiles, vs_tiles, last_exp, last_vs, ob, rep))
                )
            while pending_s4:
                fn, args = pending_s4.pop(0)
                fn(*args)
            # ---- tail: sync-nop chain so the auto drain keeps <=1 wait ----
            for fin in [st["prev_dve"], prev_batch_exp, blob_dma, blob_dma2] + out_dmas:
                if fin is None:
                    continue
                n = nc.sync.nop()
                _add_dep_helper(n.ins, fin.ins, True, "drain pre-wait")
    return nc


def _get_nc():
    if "nc" not in _CACHE:
        _CACHE["nc"] = _build()
    return _CACHE["nc"]


def make_in_maps(x, Wq, Wk, Wv, Wh):
    x = np.asarray(x, dtype=np.float32)
    xt = np.ascontiguousarray(x.transpose(0, 2, 1))  # [B, F, S]
    wq = np.asarray(Wq, dtype=np.float32)
    wk = np.asarray(Wk, dtype=np.float32)
    wv = np.asarray(Wv, dtype=np.float32)
    wh = np.asarray(Wh, dtype=np.float32)
    base = np.zeros((128, BLOB_COLS), dtype=np.float32)
    for qc in range(4):
        base[32 * qc:32 * qc + DV, 0:F] = wh
    for rb in (0, 64):
        base[rb:rb + 64, C_WQ:C_WQ + DQ] = wq
        base[rb:rb + 64, C_WK:C_WK + DK] = wk
        base[rb:rb + 64, C_WV:C_WV + DV] = wv
    maps = []
    for i in range(NCORES):
        blob = base.copy()
        for b in range(BPC):
            rb = (b % 2) * 64
            x0 = C_XT + (b // 2) * S
            blob[rb:rb + 64, x0:x0 + S] = xt[i * BPC + b]
        maps.append({"blob": blob})
    return maps


def kernel(x, Wq, Wk, Wv, Wh):
    from concourse.bass_utils import run_bass_kernel_spmd

    nc = _get_nc()
    in_maps = make_in_maps(x, Wq, Wk, Wv, Wh)
    res = run_bass_kernel_spmd(nc, in_maps, core_ids=list(range(NCORES)))
    out = np.concatenate([res.results[i]["out"] for i in range(NCORES)], axis=0)
    return out


# revision 7
# speedup vs baseline: 23.1996x; 1.1117x over previous
"""Trainium2 Bass kernel for nn_Encoder_29454885716713.

Reference computation (per batch b of B=32, S=2048, F=64):
    q = x @ Wq; k = x @ Wk; v = x @ Wv
    a = softmax(q @ k.T, axis=0 over q)       # query-axis softmax
    out = (a @ v) @ Wh
Sharding: data-parallel over batch, 4 batches per core x 8 cores.

Kernel strategy (per core) — ACT(exp)-bound, fully batch-pipelined:
  - ONE input DMA blob [128, 4240]: Wh replicated on all four 32-row
    bands, Wq/Wk/Wv duplicated in both 64-row halves, 4 pre-transposed
    xT batches packed two-per-row-half.
  - qkv stage runs in its own 1-bank PSUM pool (pq) so it overlaps the
    previous batch's score/exp chain instead of serializing behind it;
    qkT and vnat are double-buffered across batches.
  - Per k'-tile t (16 of 128): aT_t = [128, S] in PSUM (fp32r matmuls);
    ScalarE exp with fused accum_out emits exp(aT_t) to SBUF (bf16)
    plus the query-axis softmax denominator Z[k'] (free-dim reduction;
    fp32 range makes max-subtraction unnecessary).
  - 1/Z folded into v rows (vs, bf16); h1 accumulates in ONE persistent
    PSUM bank with the four 512-wide q-chunks packed at partition bases
    0/32/64/96 (PE tile_position col steps of 32), interleaved
    tile-by-tile into the exp chain; out = h1 @ Wh uses the Wh copy on
    the matching partition band, all 16 s-tiles into one PSUM tile,
    one obuf copy, one DMA per batch.
  - PSUM budget: scores 2x[128,1024] (4 banks) + pq 2x[128,512] (2)
    + h1 bank (1) + absorber bank (1) = 8.
  - This walrus build allows only ONE sync-wait slot per ISA
    instruction.  Tiny per-engine "absorber" ops (dummy matmul / copy /
    nop), each carrying exactly one cross-engine wait, precede any
    instruction that would otherwise need two.
"""

import numpy as np

_CACHE = {}

B, S, F = 32, 2048, 64
DQ, DK, DV = 24, 24, 32
NCORES = 8
BPC = B // NCORES
NT = S // 128
NQC = S // 512

C_WHR = 0
C_WQ = 64
C_WK = 88
C_WV = 112
C_XT = 144
BLOB_COLS = C_XT + (BPC // 2) * S  # 4240


def _build(lowering=True, repeat=1):
    import concourse.bass as bass
    import concourse.mybir as mybir
    import concourse.tile as tile
    from concourse.bass import _add_dep_helper

    f32 = mybir.dt.float32
    f32r = mybir.dt.float32r
    bf16 = mybir.dt.bfloat16
    EXPF = mybir.ActivationFunctionType.Exp

    def r(ap):
        return ap.bitcast(f32r)

    nc = bass.Bass(target_bir_lowering=lowering)
    blob_h = nc.dram_tensor("blob", [128, BLOB_COLS], f32r, kind="ExternalInput")
    out_h = nc.dram_tensor("out", [BPC, S, F], f32, kind="ExternalOutput")
    out_d = out_h.ap()

    with tile.TileContext(nc) as tc:
        with (
            tc.tile_pool(name="consts", bufs=1) as consts,
            tc.tile_pool(name="qkv", bufs=2) as qkv_pool,
            tc.tile_pool(name="vnat", bufs=2) as vnat_pool,
            tc.tile_pool(name="ea", bufs=16) as ea_pool,
            tc.tile_pool(name="zz", bufs=64) as z_pool,
            tc.tile_pool(name="vs", bufs=16) as vs_pool,
            tc.tile_pool(name="h1c", bufs=2) as h1c_pool,
            tc.tile_pool(name="ob", bufs=2) as ob_pool,
            tc.tile_pool(name="scr", bufs=1) as scr_pool,
            tc.tile_pool(name="pa", bufs=2, space="PSUM") as pa_pool,
            tc.tile_pool(name="pq", bufs=2, space="PSUM") as pq_pool,
            tc.tile_pool(name="php", bufs=1, space="PSUM") as php_pool,
        ):
            blob_sb = consts.tile([128, BLOB_COLS], f32r)
            sp = C_XT + S
            blob_dma = nc.sync.dma_start(
                out=blob_sb[:, 0:sp], in_=blob_h.ap()[:, 0:sp]
            )
            blob_dma2 = nc.sync.dma_start(
                out=blob_sb[:, sp:BLOB_COLS], in_=blob_h.ap()[:, sp:BLOB_COLS]
            )

            # ---------- absorber machinery ----------
            php = php_pool.tile([128, 1024], f32)
            dve_scr = scr_pool.tile([1, 256], f32)
            act_scr = scr_pool.tile([1, 256], f32)
            ctr = {"pe": 0, "dve": 0, "act": 0}

            def pe_absorb(producer):
                c = ctr["pe"]; ctr["pe"] = c + 1
                d = nc.tensor.matmul(
                    php[32:32 + DQ, (2 * c) % 1000:(2 * c) % 1000 + 2],
                    blob_sb[64:128, 0:12].bitcast(bf16),
                    blob_sb[64:128, 0:1].bitcast(bf16),
                    start=True, stop=True, skip_group_check=True,
                    tile_position=(64, 32),
                )
                if producer is not None:
                    _add_dep_helper(d.ins, producer.ins, True, "absorb")
                return d

            def dve_absorb(producer):
                c = ctr["dve"] % 250; ctr["dve"] += 1
                d = nc.vector.memset(dve_scr[:, c + 1:c + 2], 0.0)
                _add_dep_helper(d.ins, producer.ins, True, "absorb")
                return d

            def act_absorb(producer):
                c = ctr["act"] % 250; ctr["act"] += 1
                d = nc.scalar.copy(act_scr[:, c + 1:c + 2], act_scr[:, 0:1])
                if producer is not None:
                    _add_dep_helper(d.ins, producer.ins, True, "absorb")
                return d

            def order(after, before):
                _add_dep_helper(after.ins, before.ins, False, "order")

            wfence = pe_absorb(None)  # absorbs blob-DMA-1 wait on PE
            wfence2 = None  # created lazily before batch 2's qkv
            nc.vector.memset(act_scr[:, 0:1], 0.0)
            act_absorb(None)  # ACT observes the act_scr init (DVE) once

            st = {"prev_dve": None}
            prev_batch_exp = None
            prev_exp_h = [None, None]  # persists across batches
            out_dmas = []
            wf = {"w2": None}

            def make_qkv_steps(b):
                """Emission closures for batch b's q/k/v stage (10 steps).
                Interleaved into the PREVIOUS batch's score loop so the PE
                reaches them early (PE is in-order)."""
                rb = (b % 2) * 64
                x0 = C_XT + (b // 2) * S
                xT = blob_sb[rb:rb + 64, x0:x0 + S]
                tp = (rb, 0)
                wq_sb = blob_sb[rb:rb + 64, C_WQ:C_WQ + DQ]
                wk_sb = blob_sb[rb:rb + 64, C_WK:C_WK + DK]
                wv_sb = blob_sb[rb:rb + 64, C_WV:C_WV + DV]
                state = {}

                def fence(mm, first):
                    if first:
                        if b >= 2 and wf["w2"] is None:
                            wf["w2"] = pe_absorb(blob_dma2)
                        order(mm, wf["w2"] if b >= 2 else wfence)

                def qk_step(qc, which, first=False):
                    def run():
                        if "qkT" not in state:
                            state["qkT"] = qkv_pool.tile([DV, 2 * S], f32, name="qkTb")
                        qkT = state["qkT"]
                        dst = qkT[0:DQ, 0:S] if which == 0 else qkT[0:DK, S:2 * S]
                        w_sb = wq_sb if which == 0 else wk_sb
                        dd = DQ if which == 0 else DK
                        sl = slice(qc * 512, (qc + 1) * 512)
                        p = pq_pool.tile([128, 512], f32, tag="pq")
                        mm = nc.tensor.matmul(
                            p[0:dd, 0:512], w_sb, xT[:, sl],
                            start=True, stop=True, tile_position=tp,
                        )
                        fence(mm, first)
                        da = dve_absorb(mm)
                        cp = nc.vector.tensor_copy(r(dst[:, sl]), p[0:dd, 0:512])
                        order(cp, da)
                        state["last_cp"] = cp
                    return run

                def v_step(g):
                    def run():
                        if "vnat" not in state:
                            state["vnat"] = vnat_pool.tile([128, NT * DV], f32, name="vnatb")
                        vnat = state["vnat"]
                        pvt = pq_pool.tile([128, 512], f32, tag="pq")
                        mm = None
                        for i in range(8):
                            t = 8 * g + i
                            mm = nc.tensor.matmul(
                                pvt[:, i * DV:(i + 1) * DV],
                                xT[:, t * 128:(t + 1) * 128], wv_sb,
                                start=True, stop=True, tile_position=tp,
                            )
                        da = dve_absorb(mm)
                        cp = nc.vector.tensor_copy(
                            vnat[:, g * 8 * DV:(g + 1) * 8 * DV], pvt[:, 0:8 * DV]
                        )
                        order(cp, da)
                        state["last_cp"] = cp
                    return run

                steps = [qk_step(0, 0, True), qk_step(0, 1)]
                for qc in range(1, NQC):
                    steps.append(qk_step(qc, 0))
                    steps.append(qk_step(qc, 1))
                steps.append(v_step(0))
                steps.append(v_step(1))
                return steps, state

            def make_s4_pieces(ea_tiles, vs_tiles, last_exp, last_vs, ob, rep):
                """Batch tail: final h1 tile, h1->SBUF, out matmuls (in the
                pq pool), obuf copies, out DMA.  Three pieces interleaved at
                the start of the NEXT batch's score loop."""
                state = {}

                def pieceA():
                    hfence_a = pe_absorb(last_exp)
                    hfence_d = pe_absorb(last_vs)
                    lastmm = None
                    for qc in range(NQC):
                        rb2 = 64 * (qc % 2)
                        cs = 512 * (qc // 2)
                        lastmm = nc.tensor.matmul(
                            php[rb2:rb2 + DV, cs:cs + 512],
                            vs_tiles[NT - 1],
                            ea_tiles[NT - 1][:, qc * 512:(qc + 1) * 512],
                            start=False, stop=True,
                            skip_group_check=True,
                        )
                        order(lastmm, hfence_a)
                        order(lastmm, hfence_d)
                    da = dve_absorb(lastmm)
                    h1cat = h1c_pool.tile([128, 1024], f32r)
                    cp = nc.vector.tensor_copy(h1cat[0:DV, :], php[0:DV, :])
                    order(cp, da)
                    state["h1copy"] = nc.vector.tensor_copy(
                        h1cat[64:64 + DV, :], php[64:64 + DV, :]
                    )
                    state["h1cat"] = h1cat
                    state["obuf"] = ob_pool.tile([128, NT * F], f32, name="obufb")

                def out_half(half):
                    def run():
                        h1cat = state["h1cat"]
                        obuf = state["obuf"]
                        if half == 0:
                            state["ofence"] = pe_absorb(state["h1copy"])
                        pout = pq_pool.tile([128, 512], f32, tag="pq")
                        lastmm = None
                        for qci in range(2):
                            qc = 2 * half + qci
                            rb2 = 64 * (qc % 2)
                            cs = 512 * (qc // 2)
                            wh_q = blob_sb[rb2:rb2 + DV, 0:F]
                            for si in range(4):
                                ti = 4 * qci + si
                                lastmm = nc.tensor.matmul(
                                    pout[:, ti * F:(ti + 1) * F],
                                    h1cat[rb2:rb2 + DV,
                                          cs + si * 128:cs + (si + 1) * 128],
                                    wh_q,
                                    start=True, stop=True,
                                    tile_position=(rb2, 0),
                                )
                                order(lastmm, state["ofence"])
                        da = dve_absorb(lastmm)
                        da2 = None
                        if half == 0 and len(out_dmas) >= 2:
                            # obuf slot reuse: absorb the old out-DMA's tick
                            da2 = dve_absorb(out_dmas[-2])
                        cp = nc.vector.tensor_copy(
                            obuf[:, half * 512:(half + 1) * 512], pout
                        )
                        order(cp, da)
                        if da2 is not None:
                            order(cp, da2)
                        st["prev_dve"] = cp
                        if half == 1 and rep == 0:
                            odma = nc.sync.dma_start(
                                out=ob,
                                in_=obuf.rearrange("p (t f) -> p t f", f=F),
                            )
                            out_dmas.append(odma)
                    return run

                return [pieceA, out_half(0), out_half(1)]

            batches = [(rp, bb) for rp in range(repeat) for bb in range(BPC)]
            pending = []  # deferred work: s4 pieces then next-batch qkv steps
            qkv_states = {}
            # batch 0's qkv runs inline up front
            steps0, state0 = make_qkv_steps(0)
            for s_ in steps0:
                s_()
            qkv_states[0] = state0

            for bi, (rep, b) in enumerate(batches):
                ob = out_d[b].rearrange("(t p) f -> p t f", p=128)
                qstate = qkv_states.pop(bi)
                qkT = qstate["qkT"]
                vnat = qstate["vnat"]
                qT = qkT[0:DQ, 0:S]
                kT = qkT[0:DK, S:2 * S]

                # interleave queue: leftover + next batch's qkv steps
                if bi + 1 < len(batches):
                    nsteps, nstate = make_qkv_steps(batches[bi + 1][1])
                    qkv_states[bi + 1] = nstate
                else:
                    nsteps = []

                qfence = pe_absorb(qstate["last_cp"])
                ea_tiles = []
                vs_tiles = []
                last_exp = None
                last_vs = None
                vs_muls = [None] * NT
                for t in range(NT):
                    ea = ea_pool.tile([128, S], bf16, tag="ea")
                    zp = z_pool.tile([128, 2], f32, tag="zp")
                    for h in range(2):
                        afence = (
                            pe_absorb(prev_exp_h[h])
                            if prev_exp_h[h] is not None else None
                        )
                        pa = pa_pool.tile([128, 1024], f32, tag="pa")
                        lastmm = None
                        for j in range(2):
                            qc = 2 * h + j
                            lastmm = nc.tensor.matmul(
                                pa[:, j * 512:(j + 1) * 512],
                                r(kT[:, t * 128:(t + 1) * 128]),
                                r(qT[:, qc * 512:(qc + 1) * 512]),
                                start=True, stop=True,
                            )
                            if afence is not None:
                                order(lastmm, afence)
                            if t == 0:
                                order(lastmm, qfence)
                        if h == 1 and t >= 1:
                            # interleave h1 accumulation for tile t-1, all qc
                            vf = pe_absorb(vs_muls[t - 1])
                            for qc2 in range(4):
                                rb2 = 64 * (qc2 % 2)
                                cs = 512 * (qc2 // 2)
                                hm = nc.tensor.matmul(
                                    php[rb2:rb2 + DV, cs:cs + 512],
                                    vs_tiles[t - 1],
                                    ea_tiles[t - 1][:, qc2 * 512:(qc2 + 1) * 512],
                                    start=(t - 1 == 0), stop=False,
                                    skip_group_check=True,
                                )
                                order(hm, afence)
                                order(hm, vf)
                        last_exp = nc.scalar.activation(
                            out=ea[:, h * 1024:(h + 1) * 1024],
                            in_=pa,
                            func=EXPF,
                            accum_out=zp[:, h:h + 1],
                        )
                        prev_exp_h[h] = last_exp
                    zs = z_pool.tile([128, 1], f32, tag="zs")
                    nc.vector.tensor_add(zs, zp[:, 0:1], zp[:, 1:2])
                    zi = z_pool.tile([128, 1], f32, tag="zi")
                    nc.vector.reciprocal(zi, zs)
                    vs = vs_pool.tile([128, DV], bf16, tag="vs")
                    last_vs = nc.vector.tensor_scalar_mul(
                        vs, vnat[:, t * DV:(t + 1) * DV], zi
                    )
                    ea_tiles.append(ea)
                    vs_tiles.append(vs)
                    vs_muls[t] = last_vs
                    # deferred work in this t-slot's PE slack
                    if pending:
                        pending.pop(0)()
                    elif nsteps:
                        nsteps.pop(0)()
                prev_batch_exp = last_exp
                while nsteps:
                    nsteps.pop(0)()
                pending.extend(
                    make_s4_pieces(ea_tiles, vs_tiles, last_exp, last_vs, ob, rep)
                )
            while pending:
                pending.pop(0)()
            # ---- tail: sync-nop chain so the auto drain keeps <=1 wait ----
            for fin in [st["prev_dve"], prev_batch_exp, blob_dma, blob_dma2] + out_dmas:
                if fin is None:
                    continue
                n = nc.sync.nop()
                _add_dep_helper(n.ins, fin.ins, True, "drain pre-wait")
    return nc


def _get_nc():
    if "nc" not in _CACHE:
        _CACHE["nc"] = _build()
    return _CACHE["nc"]


def make_in_maps(x, Wq, Wk, Wv, Wh):
    x = np.asarray(x, dtype=np.float32)
    xt = np.ascontiguousarray(x.transpose(0, 2, 1))  # [B, F, S]
    wq = np.asarray(Wq, dtype=np.float32)
    wk = np.asarray(Wk, dtype=np.float32)
    wv = np.asarray(Wv, dtype=np.float32)
    wh = np.asarray(Wh, dtype=np.float32)
    base = np.zeros((128, BLOB_COLS), dtype=np.float32)
    for qc in range(4):
        base[32 * qc:32 * qc + DV, 0:F] = wh
    for rb in (0, 64):
        base[rb:rb + 64, C_WQ:C_WQ + DQ] = wq
        base[rb:rb + 64, C_WK:C_WK + DK] = wk
        base[rb:rb + 64, C_WV:C_WV + DV] = wv
    maps = []
    for i in range(NCORES):
        blob = base.copy()
        for b in range(BPC):
            rb = (b % 2) * 64
            x0 = C_XT + (b // 2) * S
            blob[rb:rb + 64, x0:x0 + S] = xt[i * BPC + b]
        maps.append({"blob": blob})
    return maps


def kernel(x, Wq, Wk, Wv, Wh):
    from concourse.bass_utils import run_bass_kernel_spmd

    nc = _get_nc()
    in_maps = make_in_maps(x, Wq, Wk, Wv, Wh)
    res = run_bass_kernel_spmd(nc, in_maps, core_ids=list(range(NCORES)))
    out = np.concatenate([res.results[i]["out"] for i in range(NCORES)], axis=0)
    return out


# revision 9
# speedup vs baseline: 24.0525x; 1.0368x over previous
"""Trainium2 Bass kernel for nn_Encoder_29454885716713.

Reference computation (per batch b of B=32, S=2048, F=64):
    q = x @ Wq; k = x @ Wk; v = x @ Wv
    a = softmax(q @ k.T, axis=0 over q)       # query-axis softmax
    out = (a @ v) @ Wh
Sharding: data-parallel over batch, 4 batches per core x 8 cores.

Kernel strategy (per core) — ACT(exp)-bound, fully batch-pipelined:
  - ONE input DMA blob [128, 4240]: Wh replicated on all four 32-row
    bands, Wq/Wk/Wv duplicated in both 64-row halves, 4 pre-transposed
    xT batches packed two-per-row-half.
  - qkv stage runs in its own 1-bank PSUM pool (pq) so it overlaps the
    previous batch's score/exp chain instead of serializing behind it;
    qkT and vnat are double-buffered across batches.
  - Per k'-tile t (16 of 128): aT_t = [128, S] in PSUM (fp32r matmuls);
    ScalarE exp with fused accum_out emits exp(aT_t) to SBUF (bf16)
    plus the query-axis softmax denominator Z[k'] (free-dim reduction;
    fp32 range makes max-subtraction unnecessary).
  - 1/Z folded into v rows (vs, bf16); h1 accumulates in ONE persistent
    PSUM bank with the four 512-wide q-chunks packed at partition bases
    0/32/64/96 (PE tile_position col steps of 32), interleaved
    tile-by-tile into the exp chain; out = h1 @ Wh uses the Wh copy on
    the matching partition band, all 16 s-tiles into one PSUM tile,
    one obuf copy, one DMA per batch.
  - PSUM budget: scores 2x[128,1024] (4 banks) + pq 2x[128,512] (2)
    + h1 bank (1) + absorber bank (1) = 8.
  - This walrus build allows only ONE sync-wait slot per ISA
    instruction.  Tiny per-engine "absorber" ops (dummy matmul / copy /
    nop), each carrying exactly one cross-engine wait, precede any
    instruction that would otherwise need two.
"""

import numpy as np

_CACHE = {}

B, S, F = 32, 2048, 64
DQ, DK, DV = 24, 24, 32
NCORES = 8
BPC = B // NCORES
NT = S // 128
NQC = S // 512

C_WHR = 0
C_WQ = 64
C_WK = 88
C_WV = 112
C_XT = 144
BLOB_COLS = C_XT + (BPC // 2) * S  # 4240


def _build(lowering=True, repeat=1):
    import concourse.bass as bass
    import concourse.mybir as mybir
    import concourse.tile as tile
    from concourse.bass import _add_dep_helper

    f32 = mybir.dt.float32
    f32r = mybir.dt.float32r
    bf16 = mybir.dt.bfloat16
    EXPF = mybir.ActivationFunctionType.Exp

    def r(ap):
        return ap.bitcast(f32r)

    nc = bass.Bass(target_bir_lowering=lowering)
    blob_h = nc.dram_tensor("blob", [128, BLOB_COLS], f32r, kind="ExternalInput")
    out_h = nc.dram_tensor("out", [BPC, S, F], f32, kind="ExternalOutput")
    out_d = out_h.ap()

    with tile.TileContext(nc) as tc:
        with (
            tc.tile_pool(name="consts", bufs=1) as consts,
            tc.tile_pool(name="qkv", bufs=2) as qkv_pool,
            tc.tile_pool(name="vnat", bufs=2) as vnat_pool,
            tc.tile_pool(name="ea", bufs=16) as ea_pool,
            tc.tile_pool(name="zz", bufs=64) as z_pool,
            tc.tile_pool(name="vs", bufs=16) as vs_pool,
            tc.tile_pool(name="h1c", bufs=2) as h1c_pool,
            tc.tile_pool(name="ob", bufs=2) as ob_pool,
            tc.tile_pool(name="scr", bufs=1) as scr_pool,
            tc.tile_pool(name="pa", bufs=2, space="PSUM") as pa_pool,
            tc.tile_pool(name="pq", bufs=2, space="PSUM") as pq_pool,
            tc.tile_pool(name="php", bufs=1, space="PSUM") as php_pool,
        ):
            blob_sb = consts.tile([128, BLOB_COLS], f32r)
            sp = C_XT + S
            blob_dma = nc.sync.dma_start(
                out=blob_sb[:, 0:sp], in_=blob_h.ap()[:, 0:sp]
            )
            blob_dma2 = nc.sync.dma_start(
                out=blob_sb[:, sp:BLOB_COLS], in_=blob_h.ap()[:, sp:BLOB_COLS]
            )

            # ---------- absorber machinery ----------
            php = php_pool.tile([128, 1024], f32)
            dve_scr = scr_pool.tile([1, 256], f32)
            act_scr = scr_pool.tile([1, 256], f32)
            ctr = {"pe": 0, "dve": 0, "act": 0}

            def pe_absorb(producer):
                ctr["pe"] += 1
                d = nc.tensor.ldweights(blob_sb[64:128, 0:2].bitcast(bf16))
                if producer is not None:
                    _add_dep_helper(d.ins, producer.ins, True, "absorb")
                return d

            def dve_absorb(producer):
                c = ctr["dve"] % 250; ctr["dve"] += 1
                d = nc.vector.memset(dve_scr[:, c + 1:c + 2], 0.0)
                _add_dep_helper(d.ins, producer.ins, True, "absorb")
                return d

            def act_absorb(producer):
                c = ctr["act"] % 250; ctr["act"] += 1
                d = nc.scalar.copy(act_scr[:, c + 1:c + 2], act_scr[:, 0:1])
                if producer is not None:
                    _add_dep_helper(d.ins, producer.ins, True, "absorb")
                return d

            def order(after, before):
                _add_dep_helper(after.ins, before.ins, False, "order")

            wfence = pe_absorb(None)  # absorbs blob-DMA-1 wait on PE
            wfence2 = None  # created lazily before batch 2's qkv
            nc.vector.memset(act_scr[:, 0:1], 0.0)
            act_absorb(None)  # ACT observes the act_scr init (DVE) once

            st = {"prev_dve": None}
            prev_batch_exp = None
            prev_exp_h = [None, None]  # persists across batches
            out_dmas = []
            wf = {"w2": None}

            def make_qkv_steps(b):
                """Emission closures for batch b's q/k/v stage (10 steps).
                Interleaved into the PREVIOUS batch's score loop so the PE
                reaches them early (PE is in-order)."""
                rb = (b % 2) * 64
                x0 = C_XT + (b // 2) * S
                xT = blob_sb[rb:rb + 64, x0:x0 + S]
                tp = (rb, 0)
                wq_sb = blob_sb[rb:rb + 64, C_WQ:C_WQ + DQ]
                wk_sb = blob_sb[rb:rb + 64, C_WK:C_WK + DK]
                wv_sb = blob_sb[rb:rb + 64, C_WV:C_WV + DV]
                state = {}

                def fence(mm, first):
                    if first:
                        if b >= 2 and wf["w2"] is None:
                            wf["w2"] = pe_absorb(blob_dma2)
                        order(mm, wf["w2"] if b >= 2 else wfence)

                def qk_step(qc, which, first=False):
                    def run():
                        if "qkT" not in state:
                            state["qkT"] = qkv_pool.tile([DV, 2 * S], f32, name="qkTb")
                        qkT = state["qkT"]
                        dst = qkT[0:DQ, 0:S] if which == 0 else qkT[0:DK, S:2 * S]
                        w_sb = wq_sb if which == 0 else wk_sb
                        dd = DQ if which == 0 else DK
                        sl = slice(qc * 512, (qc + 1) * 512)
                        p = pq_pool.tile([128, 512], f32, tag="pq")
                        mm = nc.tensor.matmul(
                            p[0:dd, 0:512], w_sb, xT[:, sl],
                            start=True, stop=True, tile_position=tp,
                        )
                        fence(mm, first)
                        da = dve_absorb(mm)
                        cp = nc.vector.tensor_copy(r(dst[:, sl]), p[0:dd, 0:512])
                        order(cp, da)
                        state["last_cp"] = cp
                    return run

                def v_step(g):
                    def run():
                        if "vnat" not in state:
                            state["vnat"] = vnat_pool.tile([128, NT * DV], f32, name="vnatb")
                        vnat = state["vnat"]
                        pvt = pq_pool.tile([128, 512], f32, tag="pq")
                        mm = None
                        for i in range(8):
                            t = 8 * g + i
                            mm = nc.tensor.matmul(
                                pvt[:, i * DV:(i + 1) * DV],
                                xT[:, t * 128:(t + 1) * 128], wv_sb,
                                start=True, stop=True, tile_position=tp,
                            )
                        da = dve_absorb(mm)
                        cp = nc.vector.tensor_copy(
                            vnat[:, g * 8 * DV:(g + 1) * 8 * DV], pvt[:, 0:8 * DV]
                        )
                        order(cp, da)
                        state["last_cp"] = cp
                    return run

                steps = [qk_step(0, 0, True), qk_step(0, 1)]
                for qc in range(1, NQC):
                    steps.append(qk_step(qc, 0))
                    steps.append(qk_step(qc, 1))
                steps.append(v_step(0))
                steps.append(v_step(1))
                return steps, state

            def make_s4_pieces(ea_tiles, vs_tiles, last_exp, last_vs, ob, rep):
                """Batch tail: final h1 tile, h1->SBUF, out matmuls (in the
                pq pool), obuf copies, out DMA.  Three pieces interleaved at
                the start of the NEXT batch's score loop."""
                state = {}

                def pieceA():
                    hfence_a = pe_absorb(last_exp)
                    hfence_d = pe_absorb(last_vs)
                    lastmm = None
                    for qc in range(NQC):
                        rb2 = 64 * (qc % 2)
                        cs = 512 * (qc // 2)
                        lastmm = nc.tensor.matmul(
                            php[rb2:rb2 + DV, cs:cs + 512],
                            vs_tiles[NT - 1],
                            ea_tiles[NT - 1][:, qc * 512:(qc + 1) * 512],
                            start=False, stop=True,
                            skip_group_check=True,
                        )
                        order(lastmm, hfence_a)
                        order(lastmm, hfence_d)
                    da = dve_absorb(lastmm)
                    h1cat = h1c_pool.tile([128, 1024], f32r)
                    cp = nc.vector.tensor_copy(h1cat[0:DV, :], php[0:DV, :])
                    order(cp, da)
                    state["h1copy"] = nc.vector.tensor_copy(
                        h1cat[64:64 + DV, :], php[64:64 + DV, :]
                    )
                    state["h1cat"] = h1cat
                    state["obuf"] = ob_pool.tile([128, NT * F], f32, name="obufb")

                def out_half(half):
                    def run():
                        h1cat = state["h1cat"]
                        obuf = state["obuf"]
                        if half == 0:
                            state["ofence"] = pe_absorb(state["h1copy"])
                        pout = pq_pool.tile([128, 512], f32, tag="pq")
                        lastmm = None
                        for qci in range(2):
                            qc = 2 * half + qci
                            rb2 = 64 * (qc % 2)
                            cs = 512 * (qc // 2)
                            wh_q = blob_sb[rb2:rb2 + DV, 0:F]
                            for si in range(4):
                                ti = 4 * qci + si
                                lastmm = nc.tensor.matmul(
                                    pout[:, ti * F:(ti + 1) * F],
                                    h1cat[rb2:rb2 + DV,
                                          cs + si * 128:cs + (si + 1) * 128],
                                    wh_q,
                                    start=True, stop=True,
                                    tile_position=(rb2, 0),
                                )
                                order(lastmm, state["ofence"])
                        da = dve_absorb(lastmm)
                        da2 = None
                        if half == 0 and len(out_dmas) >= 2:
                            # obuf slot reuse: absorb the old out-DMA's tick
                            da2 = dve_absorb(out_dmas[-2])
                        cp = nc.vector.tensor_copy(
                            obuf[:, half * 512:(half + 1) * 512], pout
                        )
                        order(cp, da)
                        if da2 is not None:
                            order(cp, da2)
                        st["prev_dve"] = cp
                        if half == 1 and rep == 0:
                            odma = nc.sync.dma_start(
                                out=ob,
                                in_=obuf.rearrange("p (t f) -> p t f", f=F),
                            )
                            out_dmas.append(odma)
                    return run

                return [pieceA, out_half(0), out_half(1)]

            batches = [(rp, bb) for rp in range(repeat) for bb in range(BPC)]
            pending = []  # deferred work: s4 pieces then next-batch qkv steps
            qkv_states = {}
            # batch 0's qkv runs inline up front
            steps0, state0 = make_qkv_steps(0)
            for s_ in steps0:
                s_()
            qkv_states[0] = state0

            for bi, (rep, b) in enumerate(batches):
                ob = out_d[b].rearrange("(t p) f -> p t f", p=128)
                qstate = qkv_states.pop(bi)
                qkT = qstate["qkT"]
                vnat = qstate["vnat"]
                qT = qkT[0:DQ, 0:S]
                kT = qkT[0:DK, S:2 * S]

                # interleave queue: leftover + next batch's qkv steps
                if bi + 1 < len(batches):
                    nsteps, nstate = make_qkv_steps(batches[bi + 1][1])
                    qkv_states[bi + 1] = nstate
                else:
                    nsteps = []

                qfence = pe_absorb(qstate["last_cp"])
                ea_tiles = []
                vs_tiles = []
                last_exp = None
                last_vs = None
                vs_muls = [None] * NT
                for t in range(NT):
                    ea = ea_pool.tile([128, S], bf16, tag="ea")
                    zp = z_pool.tile([128, 2], f32, tag="zp")
                    for h in range(2):
                        afence = (
                            pe_absorb(prev_exp_h[h])
                            if prev_exp_h[h] is not None else None
                        )
                        pa = pa_pool.tile([128, 1024], f32, tag="pa")
                        lastmm = None
                        for j in range(2):
                            qc = 2 * h + j
                            lastmm = nc.tensor.matmul(
                                pa[:, j * 512:(j + 1) * 512],
                                r(kT[:, t * 128:(t + 1) * 128]),
                                r(qT[:, qc * 512:(qc + 1) * 512]),
                                start=True, stop=True,
                            )
                            if afence is not None:
                                order(lastmm, afence)
                            if t == 0:
                                order(lastmm, qfence)
                        if h == 1 and t >= 1:
                            # interleave h1 accumulation for tile t-1, all qc
                            vf = pe_absorb(vs_muls[t - 1])
                            for qc2 in range(4):
                                rb2 = 64 * (qc2 % 2)
                                cs = 512 * (qc2 // 2)
                                hm = nc.tensor.matmul(
                                    php[rb2:rb2 + DV, cs:cs + 512],
                                    vs_tiles[t - 1],
                                    ea_tiles[t - 1][:, qc2 * 512:(qc2 + 1) * 512],
                                    start=(t - 1 == 0), stop=False,
                                    skip_group_check=True,
                                )
                                order(hm, afence)
                                order(hm, vf)
                        last_exp = nc.scalar.activation(
                            out=ea[:, h * 1024:(h + 1) * 1024],
                            in_=pa,
                            func=EXPF,
                            accum_out=zp[:, h:h + 1],
                        )
                        prev_exp_h[h] = last_exp
                    zs = z_pool.tile([128, 1], f32, tag="zs")
                    nc.vector.tensor_add(zs, zp[:, 0:1], zp[:, 1:2])
                    zi = z_pool.tile([128, 1], f32, tag="zi")
                    nc.vector.reciprocal(zi, zs)
                    vs = vs_pool.tile([128, DV], bf16, tag="vs")
                    last_vs = nc.vector.tensor_scalar_mul(
                        vs, vnat[:, t * DV:(t + 1) * DV], zi
                    )
                    ea_tiles.append(ea)
                    vs_tiles.append(vs)
                    vs_muls[t] = last_vs
                    # deferred work in this t-slot's PE slack
                    if pending:
                        pending.pop(0)()
                    elif nsteps:
                        nsteps.pop(0)()
                prev_batch_exp = last_exp
                while nsteps:
                    nsteps.pop(0)()
                pending.extend(
                    make_s4_pieces(ea_tiles, vs_tiles, last_exp, last_vs, ob, rep)
                )
            while pending:
                pending.pop(0)()
            # ---- tail: sync-nop chain so the auto drain keeps <=1 wait ----
            for fin in [st["prev_dve"], prev_batch_exp, blob_dma, blob_dma2] + out_dmas:
                if fin is None:
                    continue
                n = nc.sync.nop()
                _add_dep_helper(n.ins, fin.ins, True, "drain pre-wait")
    return nc


def _get_nc():
    if "nc" not in _CACHE:
        _CACHE["nc"] = _build()
    return _CACHE["nc"]


def make_in_maps(x, Wq, Wk, Wv, Wh):
    x = np.asarray(x, dtype=np.float32)
    xt = np.ascontiguousarray(x.transpose(0, 2, 1))  # [B, F, S]
    wq = np.asarray(Wq, dtype=np.float32)
    wk = np.asarray(Wk, dtype=np.float32)
    wv = np.asarray(Wv, dtype=np.float32)
    wh = np.asarray(Wh, dtype=np.float32)
    base = np.zeros((128, BLOB_COLS), dtype=np.float32)
    for qc in range(4):
        base[32 * qc:32 * qc + DV, 0:F] = wh
    for rb in (0, 64):
        base[rb:rb + 64, C_WQ:C_WQ + DQ] = wq
        base[rb:rb + 64, C_WK:C_WK + DK] = wk
        base[rb:rb + 64, C_WV:C_WV + DV] = wv
    maps = []
    for i in range(NCORES):
        blob = base.copy()
        for b in range(BPC):
            rb = (b % 2) * 64
            x0 = C_XT + (b // 2) * S
            blob[rb:rb + 64, x0:x0 + S] = xt[i * BPC + b]
        maps.append({"blob": blob})
    return maps


def kernel(x, Wq, Wk, Wv, Wh):
    from concourse.bass_utils import run_bass_kernel_spmd

    nc = _get_nc()
    in_maps = make_in_maps(x, Wq, Wk, Wv, Wh)
    res = run_bass_kernel_spmd(nc, in_maps, core_ids=list(range(NCORES)))
    out = np.concatenate([res.results[i]["out"] for i in range(NCORES)], axis=0)
    return out
